# revision 24
# baseline (speedup 1.0000x reference)
"""Multi-head causal attention block on 8 TRN2 NeuronCores.

Sharding: batch b = core//4 (2 groups of 4 cores), heads = 4*(core%4)..+3
within the group (tensor parallel over heads). Host pre-slices/permutes/
bf16-casts the weights and pre-transposes X.

Per core (4 heads, processed as 2 pairs):
  Q^T, K^T = wqk^T @ X^T   [head-pair packed: rows 0:64 head even, 64:128 odd]
  V_aug    = X @ [Wv|0]+[bv|1]  (65th col per head is constant 1 -> rowsums)
  scores^T(kb) = K_h^T.T @ Q_h^T  ->  [k, q] layout, 64-partition matmuls,
      diag block causal-masked via a PE matmul (pre-added -1e9)
  probs^T = exp(scores^T/8) via ACT straight into SBUF (no transpose needed,
      no accum readout)
  attn[q,ch]+rowsum = sum_j probsT_j^T @ V_aug_j   (PSUM accumulation; the
      ones column of V_aug accumulates the softmax denominator)
  eviction: attn * (1/rowsum) via DVE tensor_scalar (per-q scalar)
  merged^T via small PE transposes of [128q, 64ch] tiles
  A2A within the 4-core group (no zero padding, half the payload of a
      global A2A), then 2-pass projection + bias, DMA out [512, 1024] f32.
"""

import os
import sys

import numpy as np

if "/opt/trn_rl_repo" not in sys.path:
    sys.path.insert(0, "/opt/trn_rl_repo")

S = 2048
D = 1024
H = 16
HD = 64
NCORES = 8
SQ = S // 4   # rows of output per core
NKB = S // 128  # 16 k/q blocks per head

_NC_CACHE = {}


def _build_nc(debug_taps=False):
    import concourse.bass as bass
    import concourse.mybir as mybir
    import concourse.tile as tile
    from concourse import bacc
    from concourse.masks import make_identity

    f32 = mybir.dt.float32
    bf16 = mybir.dt.bfloat16

    nc = bacc.Bacc("TRN2", target_bir_lowering=False, debug=False,
                   num_devices=NCORES)

    xt_p = nc.dram_tensor("xt", [D, S], bf16, kind="ExternalInput")
    wqk_p = nc.dram_tensor("wqk", [D, 512], bf16, kind="ExternalInput")
    wv_p = nc.dram_tensor("wv", [D, 264], bf16, kind="ExternalInput")
    wp_p = nc.dram_tensor("wp", [D, D], bf16, kind="ExternalInput")
    bqk_p = nc.dram_tensor("bqk", [128, 4], f32, kind="ExternalInput")
    bv_p = nc.dram_tensor("bv", [1, 264], f32, kind="ExternalInput")
    bp_p = nc.dram_tensor("bp", [1, D], f32, kind="ExternalInput")
    gm_p = nc.dram_tensor("gmask", [128, 8], f32, kind="ExternalInput")
    out_p = nc.dram_tensor("out", [SQ, D], f32, kind="ExternalOutput")
    dbg = {}
    if debug_taps:
        dbg["qk"] = nc.dram_tensor("dbg_qk", [4, 128, S], bf16,
                                   kind="ExternalOutput")
        dbg["v"] = nc.dram_tensor("dbg_v", [128, 16 * 264], bf16,
                                  kind="ExternalOutput")
        dbg["mt"] = nc.dram_tensor("dbg_mt", [2, 128, S], bf16,
                                   kind="ExternalOutput")
        dbg["pi"] = nc.dram_tensor("dbg_pi", [8, 128, 512], bf16,
                                   kind="ExternalOutput")

    EXP = mybir.ActivationFunctionType.Exp
    IDF = mybir.ActivationFunctionType.Identity

    with tile.TileContext(nc, pool_alloc_mode="queue") as tc:
        with tc.tile_pool(name="pers", bufs=1) as pers, \
             tc.tile_pool(name="dram", bufs=1, space="DRAM") as dram:
            # ---- constants ----
            ident = pers.tile([128, 128], bf16, tag="ident", name="ident")
            make_identity(nc, ident[:])
            # cmask2[p, f] = -1e9 where f > p; used as lhsT so the PE adds
            # cmask2^T[k, q] = -1e9 where k > q into the [k, q] scores tile
            cmask2 = pers.tile([128, 128], bf16, tag="cmask2", name="cmask2")
            nc.gpsimd.memset(cmask2[:], 0.0)
            nc.gpsimd.affine_select(
                out=cmask2[:], in_=cmask2[:],
                compare_op=mybir.AluOpType.is_ge, fill=-1e9, base=0,
                pattern=[[-1, 128]], channel_multiplier=1)
            bqk_sb = pers.tile([128, 4], f32, tag="bqk", name="bqk")
            bv_row = pers.tile([1, 264], f32, tag="bvr", name="bvr")
            bp_row = pers.tile([1, D], f32, tag="bpr", name="bpr")
            bv_bc = pers.tile([128, 264], f32, tag="bvb", name="bvb")
            bp_bc = pers.tile([128, D], f32, tag="bpb", name="bpb")
            gm_sb = pers.tile([128, 8], f32, tag="gm", name="gm")

            # ---- persistent big tiles ----
            # Q/K head-pair packed: rows 0:64 head 2p, rows 64:128 head 2p+1
            qps = [pers.tile([128, S], bf16, tag=f"qp{p}", name=f"qp{p}")
                   for p in range(2)]
            kps = [pers.tile([128, S], bf16, tag=f"kp{p}", name=f"kp{p}")
                   for p in range(2)]
            # V_aug: 16 s-blocks x [128, 264]; head i at cols 66i..66i+63,
            # col 66i+64 is the constant-one column, 66i+65 zero pad
            vt = pers.tile([128, 16 * 264], bf16, tag="vt", name="vt")
            mts = [pers.tile([128, S], bf16, tag=f"mt{p}", name=f"mt{p}")
                   for p in range(2)]
            pis = [pers.tile([128, 512], bf16, tag=f"pi{i}", name=f"pi{i}")
                   for i in range(8)]
            wps = [pers.tile([128, D], bf16, tag=f"wp{i}", name=f"wp{i}")
                   for i in range(8)]

            a2a_in = [dram.tile([8, 128, 512], bf16, tag=f"a2ai{p}",
                                name=f"a2ai{p}") for p in range(2)]
            a2a_out = [dram.tile([8, 128, 512], bf16, tag=f"a2ao{p}",
                                 name=f"a2ao{p}") for p in range(2)]

            # ================= phase 1: QKV =================
            with tc.tile_pool(name="ph1", bufs=1) as ph1, \
                 tc.tile_pool(name="psq", bufs=4, space="PSUM") as psq, \
                 tc.tile_pool(name="psv", bufs=2, space="PSUM") as psv:
                xts = [ph1.tile([128, S], bf16, tag=f"xt{i}", name=f"xt{i}")
                       for i in range(8)]
                wqks = [ph1.tile([128, 512], bf16, tag=f"wqk{i}",
                                 name=f"wqk{i}") for i in range(8)]
                wvs = [ph1.tile([128, 264], bf16, tag=f"wv{i}",
                                name=f"wv{i}") for i in range(8)]
                # queue order: wqk + first xt chunks feed the first QK
                # group; alternate sync/gpsimd so both DMA queues pull
                def in_eng(kb):
                    return nc.sync if kb % 2 == 0 else nc.gpsimd
                for kb in range(4):
                    in_eng(kb).dma_start(out=wqks[kb][:],
                                         in_=wqk_p[kb * 128:(kb + 1) * 128, :])
                for kb in range(4):
                    in_eng(kb).dma_start(
                        out=xts[kb][:, 0:512],
                        in_=xt_p[kb * 128:(kb + 1) * 128, 0:512])
                for kb in range(4, 8):
                    in_eng(kb).dma_start(out=wqks[kb][:],
                                         in_=wqk_p[kb * 128:(kb + 1) * 128, :])
                for kb in range(4, 8):
                    in_eng(kb).dma_start(
                        out=xts[kb][:, 0:512],
                        in_=xt_p[kb * 128:(kb + 1) * 128, 0:512])
                for n2 in range(1, 4):
                    for kb in range(8):
                        eng = nc.sync if kb % 2 == 0 else nc.gpsimd
                        eng.dma_start(
                            out=xts[kb][:, n2 * 512:(n2 + 1) * 512],
                            in_=xt_p[kb * 128:(kb + 1) * 128,
                                     n2 * 512:(n2 + 1) * 512])
                for kb in range(8):
                    nc.gpsimd.dma_start(out=wvs[kb][:],
                                        in_=wv_p[kb * 128:(kb + 1) * 128, :])
                nc.scalar.dma_start(out=bqk_sb[:], in_=bqk_p[:])
                nc.scalar.dma_start(out=bv_row[:], in_=bv_p[:])
                nc.scalar.dma_start(out=bp_row[:], in_=bp_p[:])
                nc.scalar.dma_start(out=gm_sb[:], in_=gm_p[:])
                nc.gpsimd.partition_broadcast(bv_bc[:], bv_row[:])
                nc.gpsimd.partition_broadcast(bp_bc[:], bp_row[:])

                # QK^T: m-chunk 0 -> Q pair0, 1 -> Q pair1, 2 -> K pair0,
                # 3 -> K pair1. n2==0 splits its K loop so the first matmuls
                # start after only half the first-chunk DMAs.
                qk_dst = {0: qps[0], 1: qps[1], 2: kps[0], 3: kps[1]}
                ps0 = {}
                for m in range(4):
                    ps = psq.tile([128, 512], f32, tag="q", name="q")
                    ps0[m] = ps
                    for kb in range(4):
                        nc.tensor.matmul(
                            ps[:],
                            wqks[kb][:, m * 128:(m + 1) * 128],
                            xts[kb][:, 0:512],
                            start=(kb == 0), stop=False)
                for n2 in range(4):
                    for m in range(4):
                        if n2 == 0:
                            ps = ps0[m]
                            kbs = range(4, 8)
                        else:
                            ps = psq.tile([128, 512], f32, tag="q", name="q")
                            kbs = range(8)
                        for kb in kbs:
                            nc.tensor.matmul(
                                ps[:],
                                wqks[kb][:, m * 128:(m + 1) * 128],
                                xts[kb][:, n2 * 512:(n2 + 1) * 512],
                                start=(kb == 0), stop=(kb == 7))
                        nc.scalar.activation(
                            qk_dst[m][:, n2 * 512:(n2 + 1) * 512],
                            ps[:], IDF,
                            bias=bqk_sb[:, m:m + 1], scale=1.0)

                # V_aug [s, 4*65]
                for sb in range(16):
                    psvt = psv.tile([128, 264], f32, tag="v", name="v")
                    for kb in range(8):
                        nc.tensor.matmul(
                            psvt[:],
                            xts[kb][:, sb * 128:(sb + 1) * 128],
                            wvs[kb][:],
                            start=(kb == 0), stop=(kb == 7))
                    nc.vector.tensor_add(
                        vt[:, sb * 264:(sb + 1) * 264], psvt[:], bv_bc[:])

            for kb in range(8):
                nc.sync.dma_start(out=wps[kb][:],
                                  in_=wp_p[kb * 128:(kb + 1) * 128, :])

            if debug_taps:
                for p in range(2):
                    nc.sync.dma_start(out=dbg["qk"][p], in_=qps[p][:])
                    nc.sync.dma_start(out=dbg["qk"][2 + p], in_=kps[p][:])
                nc.sync.dma_start(out=dbg["v"][:], in_=vt[:])

            # ============ phase 2+3: attention, a2a, proj ============
            with tc.tile_pool(name="probs", bufs=2) as probs_pool, \
                 tc.tile_pool(name="small", bufs=4) as small, \
                 tc.tile_pool(name="pj", bufs=8) as pj_pool, \
                 tc.tile_pool(name="stage", bufs=4) as stage_pool, \
                 tc.tile_pool(name="pssc", bufs=2, space="PSUM") as pssc, \
                 tc.tile_pool(name="psav", bufs=2, space="PSUM") as psav, \
                 tc.tile_pool(name="pstr", bufs=2, space="PSUM") as pstr:
                # flat software pipeline over steps (pr, hl, kb): scores at
                # step t, attnV at t+LAG_AV, transpose/evict at t+LAG_TR —
                # carried across head boundaries so the PE stream never
                # waits on a just-issued ACT exp or DVE eviction.
                LAG_AV = 2
                LAG_TR = 3
                steps = [(pr, hl, kb) for pr in range(2) for hl in range(2)
                         for kb in range(NKB)]
                ptiles = {}   # (pr, hl) -> {kb: probsT tile}
                asbs = {}     # (pr, hl, qb) -> normalized attn [q, ch] tile
                trs = {}      # (pr, hl) -> current transpose psum tile

                def emit_scores(pr, hl, kb):
                    rows = slice(hl * 64, hl * 64 + 64)
                    Q = qps[pr]
                    K = kps[pr]
                    qw = S - 128 * kb
                    q0 = 128 * kb
                    pT = probs_pool.tile([128, qw], bf16, tag=f"pT{kb}",
                                         name=f"pT{kb}")
                    off = 0
                    while off < qw:
                        w = min(1024, qw - off)
                        ps = pssc.tile([128, 1024], f32, tag="sc",
                                       name="sc")
                        for c0 in range(0, w, 512):
                            cw = min(512, w - c0)
                            o = off + c0
                            if o == 0:
                                # diag block: causal mask pre-added
                                nc.tensor.matmul(
                                    ps[:, 0:128], cmask2[:], ident[:],
                                    start=True, stop=False)
                                nc.tensor.matmul(
                                    ps[:, 0:128],
                                    K[rows, q0:q0 + 128],
                                    Q[rows, q0:q0 + 128],
                                    start=False, stop=True)
                                if cw > 128:
                                    nc.tensor.matmul(
                                        ps[:, 128:cw],
                                        K[rows, q0:q0 + 128],
                                        Q[rows, q0 + 128:q0 + cw],
                                        start=True, stop=True)
                            else:
                                nc.tensor.matmul(
                                    ps[:, c0:c0 + cw],
                                    K[rows, q0:q0 + 128],
                                    Q[rows, q0 + o:q0 + o + cw],
                                    start=True, stop=True)
                        nc.scalar.activation(
                            pT[:, off:off + w], ps[:, 0:w], EXP,
                            scale=0.125)
                        off += w
                    ptiles.setdefault((pr, hl), {})[kb] = pT

                def emit_av(pr, hl, qb):
                    i = 2 * pr + hl
                    pt = ptiles[(pr, hl)]
                    pa = psav.tile([128, 128], f32, tag="av", name="av")
                    for j in range(qb + 1):
                        nc.tensor.matmul(
                            pa[:, 0:66],
                            pt[j][:, (qb - j) * 128:(qb - j) * 128 + 128],
                            vt[:, j * 264 + i * 66:j * 264 + i * 66 + 66],
                            start=(j == 0), stop=(j == qb))
                    rec = small.tile([128, 1], f32, tag="rec", name="rec")
                    nc.vector.reciprocal(rec[:], pa[:, 64:65])
                    asb = small.tile([128, 64], bf16, tag="asb",
                                     name="asb")
                    nc.vector.tensor_scalar_mul(asb[:], pa[:, 0:64],
                                                rec[:])
                    asbs[(pr, hl, qb)] = asb

                def emit_tr(pr, hl, qb):
                    rows = slice(hl * 64, hl * 64 + 64)
                    gq, ql = divmod(qb, 4)
                    if ql == 0:
                        trs[(pr, hl)] = pstr.tile([128, 512], bf16,
                                                  tag="tr", name="tr")
                    tr = trs[(pr, hl)]
                    nc.tensor.transpose(tr[rows, ql * 128:(ql + 1) * 128],
                                        asbs.pop((pr, hl, qb))[:],
                                        ident[:])
                    if ql == 3:
                        nc.vector.tensor_copy(
                            mts[pr][rows, gq * 512:(gq + 1) * 512],
                            tr[rows, :])
                        if hl == 1:
                            # both heads of the pair have landed in mts for
                            # this gq -> stage for the A2A; gmask zeroes the
                            # cross-group copy so the receiver's group-half
                            # add picks the in-group block.
                            for d in (gq, gq + 4):
                                st = stage_pool.tile([128, 512], bf16,
                                                     tag="st", name="st")
                                nc.vector.tensor_scalar_mul(
                                    st[:],
                                    mts[pr][:, gq * 512:(gq + 1) * 512],
                                    gm_sb[:, d:d + 1])
                                eng = nc.sync if d % 2 == 0 else nc.scalar
                                eng.dma_start(out=a2a_in[pr][d], in_=st[:])
                            if qb == NKB - 1:
                                nc.gpsimd.collective_compute(
                                    "AllToAll",
                                    mybir.AluOpType.bypass,
                                    replica_groups=[list(range(NCORES))],
                                    ins=[a2a_in[pr][:].opt()],
                                    outs=[a2a_out[pr][:].opt()])

                # tail injections: proj pass-1 work interleaved into the
                # last attention steps so it overlaps the A2A#1 window
                def recv_pair(pr):
                    # receiver: sum the two group halves (one is zeros) on
                    # the Pool engine, which is idle during attention --
                    # putting these on DVE would block the attention
                    # eviction stream on the A2A semaphore
                    for jj in range(4):
                        ta = stage_pool.tile([128, 512], bf16, tag="st",
                                             name="ca")
                        tb = stage_pool.tile([128, 512], bf16, tag="st",
                                             name="cb")
                        nc.sync.dma_start(out=ta[:], in_=a2a_out[pr][jj])
                        nc.scalar.dma_start(out=tb[:],
                                            in_=a2a_out[pr][4 + jj])
                        nc.gpsimd.tensor_add(pis[pr * 4 + jj][:], ta[:],
                                             tb[:])

                partials = {}

                def proj_pass1(m, n):
                    pp = pssc.tile([128, 512], f32, tag="sc", name="pp")
                    for kt in range(4):
                        nc.tensor.matmul(
                            pp[:],
                            pis[kt][:, m * 128:(m + 1) * 128],
                            wps[kt][:, n * 512:(n + 1) * 512],
                            start=(kt == 0), stop=(kt == 3))
                    so = pj_pool.tile([128, 512], f32, tag="so", name="so")
                    nc.vector.tensor_add(so[:], pp[:],
                                         bp_bc[:, n * 512:(n + 1) * 512])
                    partials[(m, n)] = so

                NS = len(steps)
                inject = {}
                idx_recv0 = NS - 6
                for k, (m, n) in enumerate([(m, n) for m in range(4)
                                            for n in range(2)]):
                    inject.setdefault(NS - 5 + k, []).append(
                        (proj_pass1, (m, n)))
                for t in range(NS + LAG_TR):
                    if 0 <= t - LAG_AV < NS:
                        emit_av(*steps[t - LAG_AV])
                    if t < NS:
                        emit_scores(*steps[t])
                    if 0 <= t - LAG_TR < NS:
                        emit_tr(*steps[t - LAG_TR])
                    if t == idx_recv0:
                        recv_pair(0)
                    for fn, args in inject.get(t, []):
                        fn(*args)

                # ---- projection pass 2 (pair-1 channels) after A2A#1 ----
                recv_pair(1)
                for m in range(4):
                    for n in range(2):
                        pp = pssc.tile([128, 512], f32, tag="sc",
                                       name="pp")
                        for kt in range(4, 8):
                            nc.tensor.matmul(
                                pp[:],
                                pis[kt][:, m * 128:(m + 1) * 128],
                                wps[kt][:, n * 512:(n + 1) * 512],
                                start=(kt == 4), stop=(kt == 7))
                        so2 = stage_pool.tile([128, 512], f32,
                                              tag="so2", name="so2")
                        nc.vector.tensor_add(so2[:], pp[:],
                                             partials[(m, n)][:])
                        oeng = nc.sync if (m + n) % 2 == 0 else nc.scalar
                        oeng.dma_start(
                            out=out_p[m * 128:(m + 1) * 128,
                                      n * 512:(n + 1) * 512],
                            in_=so2[:])

                if debug_taps:
                    for p in range(2):
                        nc.sync.dma_start(out=dbg["mt"][p], in_=mts[p][:])
                    for i2 in range(8):
                        nc.sync.dma_start(out=dbg["pi"][i2], in_=pis[i2][:])

    nc.compile()
    return nc


def _get_nc(debug_taps=False):
    key = debug_taps
    if key not in _NC_CACHE:
        _NC_CACHE[key] = _build_nc(debug_taps)
    return _NC_CACHE[key]


def _prep_in_maps(hidden_state, W_attn, b_attn, W_proj, b_proj):
    import ml_dtypes
    bf16 = ml_dtypes.bfloat16

    hidden_state = np.asarray(hidden_state, dtype=np.float32)
    W_attn = np.asarray(W_attn, dtype=np.float32)
    b_attn = np.asarray(b_attn, dtype=np.float32)
    W_proj = np.asarray(W_proj, dtype=np.float32)
    b_proj = np.asarray(b_proj, dtype=np.float32)

    # W_proj row permutation: per pair p, per source core j in group:
    # heads (4j+2p, 4j+2p+1)
    row_order = []
    for p in range(2):
        for j in range(4):
            for hh in (4 * j + 2 * p, 4 * j + 2 * p + 1):
                row_order.extend(range(hh * HD, (hh + 1) * HD))
    wp_perm = np.ascontiguousarray(W_proj[row_order, :]).astype(bf16)
    bp = np.ascontiguousarray(b_proj.reshape(1, D))

    xts = [np.ascontiguousarray(hidden_state[g].T).astype(bf16)
           for g in range(2)]

    in_maps = []
    for c in range(NCORES):
        g, j = c // 4, c % 4
        heads = [4 * j + i for i in range(4)]
        # wqk cols: Q(h0),Q(h1) | Q(h2),Q(h3) | K(h0),K(h1) | K(h2),K(h3)
        wqk = np.concatenate(
            [W_attn[:, h * HD:(h + 1) * HD] for h in heads]
            + [W_attn[:, D + h * HD:D + (h + 1) * HD] for h in heads],
            axis=1).astype(bf16)
        bqk = np.concatenate(
            [b_attn[h * HD:(h + 1) * HD] for h in heads]
            + [b_attn[D + h * HD:D + (h + 1) * HD] for h in heads])
        bqk = np.ascontiguousarray(bqk.reshape(4, 128).T)  # [128, 4]
        # V augmented with a ones column per head
        wv = np.zeros((D, 264), np.float32)
        bv = np.zeros((1, 264), np.float32)
        for i, h in enumerate(heads):
            wv[:, i * 66:i * 66 + 64] = \
                W_attn[:, 2 * D + h * HD:2 * D + (h + 1) * HD]
            bv[0, i * 66:i * 66 + 64] = \
                b_attn[2 * D + h * HD:2 * D + (h + 1) * HD]
            bv[0, i * 66 + 64] = 1.0
        gmask = np.zeros((128, 8), np.float32)
        gmask[:, 4 * g:4 * g + 4] = 1.0
        in_maps.append({
            "xt": xts[g],
            "wqk": np.ascontiguousarray(wqk),
            "wv": np.ascontiguousarray(wv.astype(bf16)),
            "wp": wp_perm,
            "bqk": bqk.astype(np.float32),
            "bv": bv,
            "bp": bp,
            "gmask": gmask,
        })
    return in_maps


def _run(in_maps, debug_taps=False, trace=False, tmpdir=None):
    from concourse.bass_utils import run_bass_kernel_spmd
    nc = _get_nc(debug_taps)
    return run_bass_kernel_spmd(nc, in_maps, core_ids=list(range(NCORES)),
                                trace=trace, tmpdir=tmpdir)


def kernel(hidden_state, W_attn, b_attn, W_proj, b_proj):
    in_maps = _prep_in_maps(hidden_state, W_attn, b_attn, W_proj, b_proj)
    res = _run(in_maps, trace=bool(os.environ.get("BASS_KERNEL_TRACE")),
               tmpdir=os.environ.get("BASS_KERNEL_TRACE_DIR") or None)
    out = np.empty((2, S, D), np.float32)
    for c in range(NCORES):
        out[c // 4, (c % 4) * SQ:(c % 4 + 1) * SQ] = res.results[c]["out"]
    if res.exec_time_ns is not None:
        kernel.last_exec_time_ns = res.exec_time_ns
    return out


kernel.last_exec_time_ns = None


# revision 26
# speedup vs baseline: 1.0409x; 1.0409x over previous
"""Multi-head causal attention block on 8 TRN2 NeuronCores.

Sharding: batch b = core//4 (2 groups of 4 cores), heads = 4*(core%4)..+3
within the group (tensor parallel over heads). Host pre-slices/permutes/
bf16-casts the weights and pre-transposes X.

Per core (4 heads, processed as 2 pairs):
  Q^T, K^T = wqk^T @ X^T   [head-pair packed: rows 0:64 head even, 64:128 odd]
  V_aug    = X @ [Wv|0]+[bv|1]  (65th col per head is constant 1 -> rowsums)
  scores^T(kb) = K_h^T.T @ Q_h^T  ->  [k, q] layout, 64-partition matmuls,
      diag block causal-masked via a PE matmul (pre-added -1e9)
  probs^T = exp(scores^T/8) via ACT straight into SBUF (no transpose needed,
      no accum readout)
  attn[q,ch]+rowsum = sum_j probsT_j^T @ V_aug_j   (PSUM accumulation; the
      ones column of V_aug accumulates the softmax denominator)
  eviction: attn * (1/rowsum) via DVE tensor_scalar (per-q scalar)
  merged^T via small PE transposes of [128q, 64ch] tiles
  A2A within the 4-core group (no zero padding, half the payload of a
      global A2A), then 2-pass projection + bias, DMA out [512, 1024] f32.
"""

import os
import sys

import numpy as np

if "/opt/trn_rl_repo" not in sys.path:
    sys.path.insert(0, "/opt/trn_rl_repo")

S = 2048
D = 1024
H = 16
HD = 64
NCORES = 8
SQ = S // 4   # rows of output per core
NKB = S // 128  # 16 k/q blocks per head

_NC_CACHE = {}


def _build_nc(debug_taps=False):
    import concourse.bass as bass
    import concourse.mybir as mybir
    import concourse.tile as tile
    from concourse import bacc
    from concourse.masks import make_identity

    f32 = mybir.dt.float32
    bf16 = mybir.dt.bfloat16

    nc = bacc.Bacc("TRN2", target_bir_lowering=False, debug=False,
                   num_devices=NCORES)

    xt_p = nc.dram_tensor("xt", [D, S], bf16, kind="ExternalInput")
    wqk_p = nc.dram_tensor("wqk", [D, 512], bf16, kind="ExternalInput")
    wv_p = nc.dram_tensor("wv", [D, 264], bf16, kind="ExternalInput")
    wp_p = nc.dram_tensor("wp", [D, D], bf16, kind="ExternalInput")
    bqk_p = nc.dram_tensor("bqk", [128, 4], f32, kind="ExternalInput")
    bv_p = nc.dram_tensor("bv", [1, 264], f32, kind="ExternalInput")
    bp_p = nc.dram_tensor("bp", [1, D], f32, kind="ExternalInput")
    gm_p = nc.dram_tensor("gmask", [128, 8], f32, kind="ExternalInput")
    out_p = nc.dram_tensor("out", [SQ, D], f32, kind="ExternalOutput")
    dbg = {}
    if debug_taps:
        dbg["qk"] = nc.dram_tensor("dbg_qk", [4, 128, S], bf16,
                                   kind="ExternalOutput")
        dbg["v"] = nc.dram_tensor("dbg_v", [128, 16 * 264], bf16,
                                  kind="ExternalOutput")
        dbg["mt"] = nc.dram_tensor("dbg_mt", [2, 128, S], bf16,
                                   kind="ExternalOutput")
        dbg["pi"] = nc.dram_tensor("dbg_pi", [8, 128, 512], bf16,
                                   kind="ExternalOutput")

    EXP = mybir.ActivationFunctionType.Exp
    IDF = mybir.ActivationFunctionType.Identity

    with tile.TileContext(nc, pool_alloc_mode="queue") as tc:
        with tc.tile_pool(name="pers", bufs=1) as pers, \
             tc.tile_pool(name="dram", bufs=1, space="DRAM") as dram:
            # ---- constants ----
            ident = pers.tile([128, 128], bf16, tag="ident", name="ident")
            make_identity(nc, ident[:])
            # cmask2[p, f] = -1e9 where f > p; used as lhsT so the PE adds
            # cmask2^T[k, q] = -1e9 where k > q into the [k, q] scores tile
            cmask2 = pers.tile([128, 128], bf16, tag="cmask2", name="cmask2")
            nc.gpsimd.memset(cmask2[:], 0.0)
            nc.gpsimd.affine_select(
                out=cmask2[:], in_=cmask2[:],
                compare_op=mybir.AluOpType.is_ge, fill=-1e9, base=0,
                pattern=[[-1, 128]], channel_multiplier=1)
            bqk_sb = pers.tile([128, 4], f32, tag="bqk", name="bqk")
            bv_row = pers.tile([1, 264], f32, tag="bvr", name="bvr")
            bp_row = pers.tile([1, D], f32, tag="bpr", name="bpr")
            bv_bc = pers.tile([128, 264], f32, tag="bvb", name="bvb")
            bp_bc = pers.tile([128, D], f32, tag="bpb", name="bpb")
            gm_sb = pers.tile([128, 8], f32, tag="gm", name="gm")

            # ---- persistent big tiles ----
            # Q/K head-pair packed: rows 0:64 head 2p, rows 64:128 head 2p+1
            qps = [pers.tile([128, S], bf16, tag=f"qp{p}", name=f"qp{p}")
                   for p in range(2)]
            kps = [pers.tile([128, S], bf16, tag=f"kp{p}", name=f"kp{p}")
                   for p in range(2)]
            # V_aug: 16 s-blocks x [128, 264]; head i at cols 66i..66i+63,
            # col 66i+64 is the constant-one column, 66i+65 zero pad
            vt = pers.tile([128, 16 * 264], bf16, tag="vt", name="vt")
            mts = [pers.tile([128, S], bf16, tag=f"mt{p}", name=f"mt{p}")
                   for p in range(2)]
            pis = [pers.tile([128, 512], bf16, tag=f"pi{i}", name=f"pi{i}")
                   for i in range(8)]
            wps = [pers.tile([128, D], bf16, tag=f"wp{i}", name=f"wp{i}")
                   for i in range(8)]

            a2a_in = [dram.tile([8, 128, 512], bf16, tag=f"a2ai{p}",
                                name=f"a2ai{p}") for p in range(2)]
            a2a_out = [dram.tile([8, 128, 512], bf16, tag=f"a2ao{p}",
                                 name=f"a2ao{p}") for p in range(2)]

            # ================= phase 1: QKV =================
            with tc.tile_pool(name="ph1", bufs=1) as ph1, \
                 tc.tile_pool(name="psq", bufs=4, space="PSUM") as psq, \
                 tc.tile_pool(name="psv", bufs=2, space="PSUM") as psv:
                xts = [ph1.tile([128, S], bf16, tag=f"xt{i}", name=f"xt{i}")
                       for i in range(8)]
                wqks = [ph1.tile([128, 512], bf16, tag=f"wqk{i}",
                                 name=f"wqk{i}") for i in range(8)]
                wvs = [ph1.tile([128, 264], bf16, tag=f"wv{i}",
                                name=f"wv{i}") for i in range(8)]
                # queue order: wqk + first xt chunks feed the first QK
                # group; alternate sync/gpsimd so both DMA queues pull
                def in_eng(kb):
                    return nc.sync if kb % 2 == 0 else nc.gpsimd
                for kb in range(4):
                    in_eng(kb).dma_start(out=wqks[kb][:],
                                         in_=wqk_p[kb * 128:(kb + 1) * 128, :])
                for kb in range(4):
                    in_eng(kb).dma_start(
                        out=xts[kb][:, 0:512],
                        in_=xt_p[kb * 128:(kb + 1) * 128, 0:512])
                for kb in range(4, 8):
                    in_eng(kb).dma_start(out=wqks[kb][:],
                                         in_=wqk_p[kb * 128:(kb + 1) * 128, :])
                for kb in range(4, 8):
                    in_eng(kb).dma_start(
                        out=xts[kb][:, 0:512],
                        in_=xt_p[kb * 128:(kb + 1) * 128, 0:512])
                for n2 in range(1, 4):
                    for kb in range(8):
                        eng = nc.sync if kb % 2 == 0 else nc.gpsimd
                        eng.dma_start(
                            out=xts[kb][:, n2 * 512:(n2 + 1) * 512],
                            in_=xt_p[kb * 128:(kb + 1) * 128,
                                     n2 * 512:(n2 + 1) * 512])
                for kb in range(8):
                    nc.gpsimd.dma_start(out=wvs[kb][:],
                                        in_=wv_p[kb * 128:(kb + 1) * 128, :])
                nc.scalar.dma_start(out=bqk_sb[:], in_=bqk_p[:])
                nc.scalar.dma_start(out=bv_row[:], in_=bv_p[:])
                nc.scalar.dma_start(out=bp_row[:], in_=bp_p[:])
                nc.scalar.dma_start(out=gm_sb[:], in_=gm_p[:])
                nc.gpsimd.partition_broadcast(bv_bc[:], bv_row[:])
                nc.gpsimd.partition_broadcast(bp_bc[:], bp_row[:])

                # QK^T: m-chunk 0 -> Q pair0, 1 -> Q pair1, 2 -> K pair0,
                # 3 -> K pair1. n2==0 splits its K loop so the first matmuls
                # start after only half the first-chunk DMAs.
                qk_dst = {0: qps[0], 1: qps[1], 2: kps[0], 3: kps[1]}
                ps0 = {}
                for m in range(4):
                    ps = psq.tile([128, 512], f32, tag="q", name="q")
                    ps0[m] = ps
                    for kb in range(4):
                        nc.tensor.matmul(
                            ps[:],
                            wqks[kb][:, m * 128:(m + 1) * 128],
                            xts[kb][:, 0:512],
                            start=(kb == 0), stop=False)
                for n2 in range(4):
                    for m in range(4):
                        if n2 == 0:
                            ps = ps0[m]
                            kbs = range(4, 8)
                        else:
                            ps = psq.tile([128, 512], f32, tag="q", name="q")
                            kbs = range(8)
                        for kb in kbs:
                            nc.tensor.matmul(
                                ps[:],
                                wqks[kb][:, m * 128:(m + 1) * 128],
                                xts[kb][:, n2 * 512:(n2 + 1) * 512],
                                start=(kb == 0), stop=(kb == 7))
                        nc.scalar.activation(
                            qk_dst[m][:, n2 * 512:(n2 + 1) * 512],
                            ps[:], IDF,
                            bias=bqk_sb[:, m:m + 1], scale=1.0)

                # V_aug [s, 4*65]
                for sb in range(16):
                    psvt = psv.tile([128, 264], f32, tag="v", name="v")
                    for kb in range(8):
                        nc.tensor.matmul(
                            psvt[:],
                            xts[kb][:, sb * 128:(sb + 1) * 128],
                            wvs[kb][:],
                            start=(kb == 0), stop=(kb == 7))
                    nc.vector.tensor_add(
                        vt[:, sb * 264:(sb + 1) * 264], psvt[:], bv_bc[:])

            for kb in range(8):
                nc.sync.dma_start(out=wps[kb][:],
                                  in_=wp_p[kb * 128:(kb + 1) * 128, :])

            if debug_taps:
                for p in range(2):
                    nc.sync.dma_start(out=dbg["qk"][p], in_=qps[p][:])
                    nc.sync.dma_start(out=dbg["qk"][2 + p], in_=kps[p][:])
                nc.sync.dma_start(out=dbg["v"][:], in_=vt[:])

            # ============ phase 2+3: attention, a2a, proj ============
            with tc.tile_pool(name="probs", bufs=2) as probs_pool, \
                 tc.tile_pool(name="small", bufs=4) as small, \
                 tc.tile_pool(name="pj", bufs=8) as pj_pool, \
                 tc.tile_pool(name="stage", bufs=4) as stage_pool, \
                 tc.tile_pool(name="pssc", bufs=2, space="PSUM") as pssc, \
                 tc.tile_pool(name="pss5", bufs=1, space="PSUM") as pss5, \
                 tc.tile_pool(name="psav", bufs=2, space="PSUM") as psav, \
                 tc.tile_pool(name="pstr", bufs=1, space="PSUM") as pstr:
                # flat software pipeline over steps (pr, hl, kb): scores at
                # step t, attnV at t+LAG_AV, transpose/evict at t+LAG_TR —
                # carried across head boundaries so the PE stream never
                # waits on a just-issued ACT exp or DVE eviction.
                LAG_AV = 2
                LAG_TR = 3
                steps = [(pr, hl, kb) for pr in range(2) for hl in range(2)
                         for kb in range(NKB)]
                ptiles = {}   # (pr, hl) -> {kb: probsT tile}
                asbs = {}     # (pr, hl, qb) -> normalized attn [q, ch] tile
                trs = {}      # (pr, hl) -> current transpose psum tile

                def emit_scores(pr, hl, kb):
                    rows = slice(hl * 64, hl * 64 + 64)
                    Q = qps[pr]
                    K = kps[pr]
                    qw = S - 128 * kb
                    q0 = 128 * kb
                    pT = probs_pool.tile([128, qw], bf16, tag=f"pT{kb}",
                                         name=f"pT{kb}")
                    off = 0
                    while off < qw:
                        w = min(1024, qw - off)
                        if w <= 512:
                            ps = pss5.tile([128, 512], f32, tag="sc5",
                                           name="sc5")
                        else:
                            ps = pssc.tile([128, 1024], f32, tag="sc",
                                           name="sc")
                        for c0 in range(0, w, 512):
                            cw = min(512, w - c0)
                            o = off + c0
                            if o == 0:
                                # diag block: causal mask pre-added
                                nc.tensor.matmul(
                                    ps[:, 0:128], cmask2[:], ident[:],
                                    start=True, stop=False)
                                nc.tensor.matmul(
                                    ps[:, 0:128],
                                    K[rows, q0:q0 + 128],
                                    Q[rows, q0:q0 + 128],
                                    start=False, stop=True)
                                if cw > 128:
                                    nc.tensor.matmul(
                                        ps[:, 128:cw],
                                        K[rows, q0:q0 + 128],
                                        Q[rows, q0 + 128:q0 + cw],
                                        start=True, stop=True)
                            else:
                                nc.tensor.matmul(
                                    ps[:, c0:c0 + cw],
                                    K[rows, q0:q0 + 128],
                                    Q[rows, q0 + o:q0 + o + cw],
                                    start=True, stop=True)
                        nc.scalar.activation(
                            pT[:, off:off + w], ps[:, 0:w], EXP,
                            scale=0.125)
                        off += w
                    ptiles.setdefault((pr, hl), {})[kb] = pT

                def emit_av(pr, hl, qb):
                    i = 2 * pr + hl
                    pt = ptiles[(pr, hl)]
                    pa = psav.tile([128, 128], f32, tag="av", name="av")
                    for j in range(qb + 1):
                        nc.tensor.matmul(
                            pa[:, 0:66],
                            pt[j][:, (qb - j) * 128:(qb - j) * 128 + 128],
                            vt[:, j * 264 + i * 66:j * 264 + i * 66 + 66],
                            start=(j == 0), stop=(j == qb))
                    rec = small.tile([128, 1], f32, tag="rec", name="rec")
                    nc.vector.reciprocal(rec[:], pa[:, 64:65])
                    asb = small.tile([128, 64], bf16, tag="asb",
                                     name="asb")
                    nc.vector.tensor_scalar_mul(asb[:], pa[:, 0:64],
                                                rec[:])
                    asbs[(pr, hl, qb)] = asb

                def emit_tr(pr, hl, qb):
                    rows = slice(hl * 64, hl * 64 + 64)
                    gq, ql = divmod(qb, 4)
                    if ql == 0:
                        trs[(pr, hl)] = pstr.tile([128, 512], bf16,
                                                  tag="tr", name="tr")
                    tr = trs[(pr, hl)]
                    nc.tensor.transpose(tr[rows, ql * 128:(ql + 1) * 128],
                                        asbs.pop((pr, hl, qb))[:],
                                        ident[:])
                    if ql == 3:
                        nc.vector.tensor_copy(
                            mts[pr][rows, gq * 512:(gq + 1) * 512],
                            tr[rows, :])
                        if hl == 1:
                            # both heads of the pair have landed in mts for
                            # this gq -> stage for the A2A; gmask zeroes the
                            # cross-group copy so the receiver's group-half
                            # add picks the in-group block.
                            for d in (gq, gq + 4):
                                st = stage_pool.tile([128, 512], bf16,
                                                     tag="st", name="st")
                                nc.vector.tensor_scalar_mul(
                                    st[:],
                                    mts[pr][:, gq * 512:(gq + 1) * 512],
                                    gm_sb[:, d:d + 1])
                                eng = nc.sync if d % 2 == 0 else nc.scalar
                                eng.dma_start(out=a2a_in[pr][d], in_=st[:])
                            if qb == NKB - 1:
                                nc.gpsimd.collective_compute(
                                    "AllToAll",
                                    mybir.AluOpType.bypass,
                                    replica_groups=[list(range(NCORES))],
                                    ins=[a2a_in[pr][:].opt()],
                                    outs=[a2a_out[pr][:].opt()])

                # tail injections: proj pass-1 work interleaved into the
                # last attention steps so it overlaps the A2A#1 window
                def recv_pair(pr):
                    # receiver: sum the two group halves (one is zeros) on
                    # the Pool engine, which is idle during attention --
                    # putting these on DVE would block the attention
                    # eviction stream on the A2A semaphore
                    for jj in range(4):
                        ta = stage_pool.tile([128, 512], bf16, tag="st",
                                             name="ca")
                        tb = stage_pool.tile([128, 512], bf16, tag="st",
                                             name="cb")
                        nc.sync.dma_start(out=ta[:], in_=a2a_out[pr][jj])
                        nc.scalar.dma_start(out=tb[:],
                                            in_=a2a_out[pr][4 + jj])
                        nc.gpsimd.tensor_add(pis[pr * 4 + jj][:], ta[:],
                                             tb[:])

                partials = {}

                def proj_pass1(m, n):
                    pp = pssc.tile([128, 512], f32, tag="sc", name="pp")
                    for kt in range(4):
                        nc.tensor.matmul(
                            pp[:],
                            pis[kt][:, m * 128:(m + 1) * 128],
                            wps[kt][:, n * 512:(n + 1) * 512],
                            start=(kt == 0), stop=(kt == 3))
                    so = pj_pool.tile([128, 512], f32, tag="so", name="so")
                    nc.vector.tensor_add(so[:], pp[:],
                                         bp_bc[:, n * 512:(n + 1) * 512])
                    partials[(m, n)] = so

                NS = len(steps)
                inject = {}
                idx_recv0 = NS - 6
                for k, (m, n) in enumerate([(m, n) for m in range(4)
                                            for n in range(2)]):
                    inject.setdefault(NS - 5 + k, []).append(
                        (proj_pass1, (m, n)))
                for t in range(NS + LAG_TR):
                    if 0 <= t - LAG_AV < NS:
                        emit_av(*steps[t - LAG_AV])
                    if t < NS:
                        emit_scores(*steps[t])
                    if 0 <= t - LAG_TR < NS:
                        emit_tr(*steps[t - LAG_TR])
                    if t == idx_recv0:
                        recv_pair(0)
                    for fn, args in inject.get(t, []):
                        fn(*args)

                # ---- projection pass 2 (pair-1 channels) after A2A#1 ----
                recv_pair(1)
                for m in range(4):
                    for n in range(2):
                        pp = pssc.tile([128, 512], f32, tag="sc",
                                       name="pp")
                        for kt in range(4, 8):
                            nc.tensor.matmul(
                                pp[:],
                                pis[kt][:, m * 128:(m + 1) * 128],
                                wps[kt][:, n * 512:(n + 1) * 512],
                                start=(kt == 4), stop=(kt == 7))
                        so2 = stage_pool.tile([128, 512], f32,
                                              tag="so2", name="so2")
                        nc.vector.tensor_add(so2[:], pp[:],
                                             partials[(m, n)][:])
                        oeng = nc.sync if (m + n) % 2 == 0 else nc.scalar
                        oeng.dma_start(
                            out=out_p[m * 128:(m + 1) * 128,
                                      n * 512:(n + 1) * 512],
                            in_=so2[:])

                if debug_taps:
                    for p in range(2):
                        nc.sync.dma_start(out=dbg["mt"][p], in_=mts[p][:])
                    for i2 in range(8):
                        nc.sync.dma_start(out=dbg["pi"][i2], in_=pis[i2][:])

    nc.compile()
    return nc


def _get_nc(debug_taps=False):
    key = debug_taps
    if key not in _NC_CACHE:
        _NC_CACHE[key] = _build_nc(debug_taps)
    return _NC_CACHE[key]


def _prep_in_maps(hidden_state, W_attn, b_attn, W_proj, b_proj):
    import ml_dtypes
    bf16 = ml_dtypes.bfloat16

    hidden_state = np.asarray(hidden_state, dtype=np.float32)
    W_attn = np.asarray(W_attn, dtype=np.float32)
    b_attn = np.asarray(b_attn, dtype=np.float32)
    W_proj = np.asarray(W_proj, dtype=np.float32)
    b_proj = np.asarray(b_proj, dtype=np.float32)

    # W_proj row permutation: per pair p, per source core j in group:
    # heads (4j+2p, 4j+2p+1)
    row_order = []
    for p in range(2):
        for j in range(4):
            for hh in (4 * j + 2 * p, 4 * j + 2 * p + 1):
                row_order.extend(range(hh * HD, (hh + 1) * HD))
    wp_perm = np.ascontiguousarray(W_proj[row_order, :]).astype(bf16)
    bp = np.ascontiguousarray(b_proj.reshape(1, D))

    xts = [np.ascontiguousarray(hidden_state[g].T).astype(bf16)
           for g in range(2)]

    in_maps = []
    for c in range(NCORES):
        g, j = c // 4, c % 4
        heads = [4 * j + i for i in range(4)]
        # wqk cols: Q(h0),Q(h1) | Q(h2),Q(h3) | K(h0),K(h1) | K(h2),K(h3)
        wqk = np.concatenate(
            [W_attn[:, h * HD:(h + 1) * HD] for h in heads]
            + [W_attn[:, D + h * HD:D + (h + 1) * HD] for h in heads],
            axis=1).astype(bf16)
        bqk = np.concatenate(
            [b_attn[h * HD:(h + 1) * HD] for h in heads]
            + [b_attn[D + h * HD:D + (h + 1) * HD] for h in heads])
        bqk = np.ascontiguousarray(bqk.reshape(4, 128).T)  # [128, 4]
        # V augmented with a ones column per head
        wv = np.zeros((D, 264), np.float32)
        bv = np.zeros((1, 264), np.float32)
        for i, h in enumerate(heads):
            wv[:, i * 66:i * 66 + 64] = \
                W_attn[:, 2 * D + h * HD:2 * D + (h + 1) * HD]
            bv[0, i * 66:i * 66 + 64] = \
                b_attn[2 * D + h * HD:2 * D + (h + 1) * HD]
            bv[0, i * 66 + 64] = 1.0
        gmask = np.zeros((128, 8), np.float32)
        gmask[:, 4 * g:4 * g + 4] = 1.0
        in_maps.append({
            "xt": xts[g],
            "wqk": np.ascontiguousarray(wqk),
            "wv": np.ascontiguousarray(wv.astype(bf16)),
            "wp": wp_perm,
            "bqk": bqk.astype(np.float32),
            "bv": bv,
            "bp": bp,
            "gmask": gmask,
        })
    return in_maps


def _run(in_maps, debug_taps=False, trace=False, tmpdir=None):
    from concourse.bass_utils import run_bass_kernel_spmd
    nc = _get_nc(debug_taps)
    return run_bass_kernel_spmd(nc, in_maps, core_ids=list(range(NCORES)),
                                trace=trace, tmpdir=tmpdir)


def kernel(hidden_state, W_attn, b_attn, W_proj, b_proj):
    in_maps = _prep_in_maps(hidden_state, W_attn, b_attn, W_proj, b_proj)
    res = _run(in_maps, trace=bool(os.environ.get("BASS_KERNEL_TRACE")),
               tmpdir=os.environ.get("BASS_KERNEL_TRACE_DIR") or None)
    out = np.empty((2, S, D), np.float32)
    for c in range(NCORES):
        out[c // 4, (c % 4) * SQ:(c % 4 + 1) * SQ] = res.results[c]["out"]
    if res.exec_time_ns is not None:
        kernel.last_exec_time_ns = res.exec_time_ns
    return out


kernel.last_exec_time_ns = None
